# revision 1
# baseline (speedup 1.0000x reference)
"""Trainium2 Bass kernel for nn_ContrastiveLoss (SimCLR + spatial contrastive loss).

Strategy (8-core data parallel):
  - Host: L2-normalize z1/z2/embeddings (fp32), build transposed bf16 operand
    tables, gather anchor rows, compute fp32 positive-pair dots.
  - Device (per core): bf16 matmuls of its 1024 simclr rows and 512 spatial
    rows against the full 8192-column tables; fused exp(x/T) + row-sum on the
    ACT engine straight out of PSUM; a tiny PE Gram matmul per row-tile whose
    diagonal reproduces bit-exactly the self-similarity terms, which are
    exp'd identically and returned as per-row corrections.
  - Host: sum_exp = S_raw - corr (exact diagonal/anchor-column removal),
    log, subtract positives, mean-reduce -> [2] losses.

Self-contained: hardcodes shapes from the problem spec.
"""
import sys

for _p in ("/opt/trn_rl_repo", "/root/.axon_site/_ro/trn_rl_repo"):
    if _p not in sys.path:
        sys.path.insert(0, _p)

import numpy as np
import ml_dtypes

import concourse.tile as tile
from concourse import bacc, mybir
from concourse.bass_utils import run_bass_kernel_spmd

TEMPERATURE = 0.07
B = 4096     # simclr batch
D = 256      # projection dim
N = 8192     # num cells (spatial table rows, also 2B simclr table rows)
P = 4096     # num spatial pairs
NCORES = 8
SR = B // NCORES          # 512 simclr pair-rows per core (=> 1024 sim rows)
PR = P // NCORES          # 512 spatial rows per core
RT_SIMCLR = (2 * SR) // 128   # 8 row-tiles
RT_SPATIAL = PR // 128        # 4 row-tiles
RT_TOTAL = RT_SIMCLR + RT_SPATIAL  # 12
NCHUNK = N // 512         # 16 column chunks of 512
NGROUP = 4                # psum groups of 2048 columns
F32 = mybir.dt.float32
BF16 = mybir.dt.bfloat16

_CACHE = {}


def _build_nc():
    nc = bacc.Bacc("TRN2", target_bir_lowering=False)

    zT = nc.dram_tensor("zT", [128, 2, N], BF16, kind="ExternalInput")
    eT = nc.dram_tensor("eT", [128, 2, N], BF16, kind="ExternalInput")
    zTl = nc.dram_tensor("zTl", [128, 2, 2 * SR], BF16, kind="ExternalInput")
    aTl = nc.dram_tensor("aTl", [128, 2, PR], BF16, kind="ExternalInput")
    ident = nc.dram_tensor("ident", [128, 128], F32, kind="ExternalInput")

    sraw_o = nc.dram_tensor("sraw", [128, RT_TOTAL], F32, kind="ExternalOutput")
    corr_o = nc.dram_tensor("corr", [128, RT_TOTAL], F32, kind="ExternalOutput")

    inv_t = float(1.0 / np.float32(TEMPERATURE))

    with tile.TileContext(nc) as tc:
        with (
            tc.tile_pool(name="tabs", bufs=1) as tabs,
            tc.tile_pool(name="psum", bufs=2, space="PSUM") as psum,
            tc.tile_pool(name="scr", bufs=3) as scrp,
            tc.tile_pool(name="small", bufs=1) as small,
            tc.tile_pool(name="tmp", bufs=4) as tmpp,
        ):
            zT_t = tabs.tile([128, 2, N], BF16)
            eT_t = tabs.tile([128, 2, N], BF16)
            zTl_t = tabs.tile([128, 2, 2 * SR], BF16)
            aTl_t = tabs.tile([128, 2, PR], BF16)
            ident_t = small.tile([128, 128], F32)
            nc.sync.dma_start(zT_t[:], zT[:])
            nc.sync.dma_start(eT_t[:], eT[:])
            nc.sync.dma_start(zTl_t[:], zTl[:])
            nc.sync.dma_start(aTl_t[:], aTl[:])
            nc.sync.dma_start(ident_t[:], ident[:])

            sraw_t = small.tile([128, RT_TOTAL], F32)
            corr_t = small.tile([128, RT_TOTAL], F32)

            for rt in range(RT_TOTAL):
                if rt < RT_SIMCLR:
                    lh, li, tab = zTl_t, rt, zT_t
                else:
                    lh, li, tab = aTl_t, rt - RT_SIMCLR, eT_t
                lhsT0 = lh[:, 0, li * 128:(li + 1) * 128]
                lhsT1 = lh[:, 1, li * 128:(li + 1) * 128]

                part = tmpp.tile([128, NGROUP], F32, tag="part")
                for g in range(NGROUP):
                    pg = psum.tile([128, 2048], F32, tag="big")
                    for kc, lz in ((0, lhsT0), (1, lhsT1)):
                        for cc in range(4):
                            col = g * 2048 + cc * 512
                            nc.tensor.matmul(
                                pg[:, cc * 512:(cc + 1) * 512],
                                lz,
                                tab[:, kc, col:col + 512],
                                start=(kc == 0),
                                stop=(kc == 1),
                            )
                    scr = scrp.tile([128, 2048], F32, tag="scr")
                    nc.scalar.activation(
                        scr[:], pg[:], mybir.ActivationFunctionType.Exp,
                        scale=inv_t, accum_out=part[:, g:g + 1],
                    )
                nc.vector.tensor_reduce(
                    sraw_t[:, rt:rt + 1], part[:],
                    axis=mybir.AxisListType.X, op=mybir.AluOpType.add,
                )

                # Gram diagonal: bitwise-identical to the main matmul's
                # self-similarity element for each row.
                pgr = psum.tile([128, 2048], F32, tag="big")
                nc.tensor.matmul(pgr[:, :128], lhsT0, lhsT0, start=True, stop=False)
                nc.tensor.matmul(pgr[:, :128], lhsT1, lhsT1, start=False, stop=True)
                gd = tmpp.tile([128, 128], F32, tag="gd")
                nc.vector.tensor_tensor(
                    gd[:], pgr[:, :128], ident_t[:], mybir.AluOpType.mult
                )
                gdv = tmpp.tile([128, 1], F32, tag="gdv")
                nc.vector.tensor_reduce(
                    gdv[:], gd[:], axis=mybir.AxisListType.X, op=mybir.AluOpType.add
                )
                nc.scalar.activation(
                    corr_t[:, rt:rt + 1], gdv[:],
                    mybir.ActivationFunctionType.Exp, scale=inv_t,
                )

            nc.sync.dma_start(sraw_o[:], sraw_t[:])
            nc.sync.dma_start(corr_o[:], corr_t[:])

    nc.finalize()
    return nc


def _l2norm(x):
    n = np.maximum(np.linalg.norm(x.astype(np.float32), axis=1, keepdims=True), 1e-12)
    return (x.astype(np.float32) / n).astype(np.float32)


def _pack_T(x):
    """[R, D=256] fp32 -> transposed bf16 operand table [128, 2, R]."""
    xT = np.ascontiguousarray(x.T)                      # [256, R]
    return np.ascontiguousarray(
        xT.reshape(2, 128, xT.shape[1]).transpose(1, 0, 2)
    ).astype(ml_dtypes.bfloat16)


def prepare(z1, z2, embeddings, anchor_idx, neighbor_idx):
    """Host-side prep: returns (in_maps, host_ctx)."""
    z1n = _l2norm(np.asarray(z1))
    z2n = _l2norm(np.asarray(z2))
    en = _l2norm(np.asarray(embeddings))
    ai = np.asarray(anchor_idx).astype(np.int64)
    ni = np.asarray(neighbor_idx).astype(np.int64)

    zcat = np.concatenate([z1n, z2n], axis=0)           # [2B, D]
    zT_p = _pack_T(zcat)                                # [128, 2, 8192] bf16
    eT_p = _pack_T(en)                                  # [128, 2, 8192] bf16
    a_rows = en[ai]                                     # [P, D] fp32
    aT_p = _pack_T(a_rows)                              # [128, 2, 4096] bf16

    # fp32 positive-pair logits (match reference semantics)
    psim = (np.sum(z1n.astype(np.float64) * z2n.astype(np.float64), axis=1)
            / np.float64(np.float32(TEMPERATURE)))      # [B]
    pos = (np.sum(a_rows.astype(np.float64) * en[ni].astype(np.float64), axis=1)
           / np.float64(np.float32(TEMPERATURE)))       # [P]
    eq = (ai == ni).astype(np.float64)                  # [P]

    ident = np.eye(128, dtype=np.float32)
    in_maps = []
    for c in range(NCORES):
        zTl_p = np.ascontiguousarray(np.concatenate(
            [zT_p[:, :, c * SR:(c + 1) * SR],
             zT_p[:, :, B + c * SR:B + (c + 1) * SR]], axis=2))  # [128,2,1024]
        aTl_p = np.ascontiguousarray(aT_p[:, :, c * PR:(c + 1) * PR])  # [128,2,512]
        in_maps.append({
            "zT": zT_p, "eT": eT_p, "zTl": zTl_p, "aTl": aTl_p, "ident": ident,
        })
    return in_maps, (psim, pos, eq)


def finish(results, host_ctx):
    """Host-side epilogue: assemble the two losses from per-core S_raw/corr."""
    psim, pos, eq = host_ctx
    terms1 = np.empty(2 * B, dtype=np.float64)
    terms2 = np.empty(P, dtype=np.float64)
    for c in range(NCORES):
        S = results[c]["sraw"].astype(np.float64).T.reshape(-1)   # [12*128], idx rt*128+p
        C = results[c]["corr"].astype(np.float64).T.reshape(-1)

        s_sim = S[:2 * SR * 1]  # first 8 tiles = 1024 rows
        c_sim = C[:2 * SR]
        sum_exp = s_sim[:2 * SR] - c_sim[:2 * SR]
        p_loc = psim[c * SR:(c + 1) * SR]
        # local rows [0,512) -> z1 part, [512,1024) -> z2 part; same positives
        terms1[c * SR:(c + 1) * SR] = np.log(sum_exp[:SR]) - p_loc
        terms1[B + c * SR:B + (c + 1) * SR] = np.log(sum_exp[SR:2 * SR]) - p_loc

        s_sp = S[2 * SR:2 * SR + PR]
        c_sp = C[2 * SR:2 * SR + PR]
        g = slice(c * PR, (c + 1) * PR)
        total = s_sp - c_sp + eq[g] * np.exp(pos[g])
        terms2[g] = np.log(total) - pos[g]

    l1 = terms1.mean()
    l2 = terms2.mean()
    return np.array([l1, l2], dtype=np.float32)


def get_nc():
    if "nc" not in _CACHE:
        _CACHE["nc"] = _build_nc()
    return _CACHE["nc"]


def kernel(z1, z2, embeddings, anchor_idx, neighbor_idx):
    in_maps, host_ctx = prepare(z1, z2, embeddings, anchor_idx, neighbor_idx)
    nc = get_nc()
    res = run_bass_kernel_spmd(nc, in_maps, list(range(NCORES)))
    return finish(res.results, host_ctx)


# revision 8
# speedup vs baseline: 1.2647x; 1.2647x over previous
"""Trainium2 Bass kernel for nn_ContrastiveLoss (SimCLR + spatial contrastive loss).

Strategy (8-core data parallel):
  - Host: L2-normalize z1/z2/embeddings (fp32), build transposed bf16 operand
    tables, gather anchor rows, compute fp32 positive-pair dots.
  - Device (per core): bf16 matmuls of its 1024 simclr rows and 512 spatial
    rows against the full 8192-column tables; fused exp(x/T) + row-sum on the
    ACT engine straight out of PSUM; a tiny PE Gram matmul per row-tile whose
    diagonal reproduces bit-exactly the self-similarity terms, which are
    exp'd identically and returned as per-row corrections.
  - Host: sum_exp = S_raw - corr (exact diagonal/anchor-column removal),
    log, subtract positives, mean-reduce -> [2] losses.

Self-contained: hardcodes shapes from the problem spec.
"""
import sys

for _p in ("/opt/trn_rl_repo", "/root/.axon_site/_ro/trn_rl_repo"):
    if _p not in sys.path:
        sys.path.insert(0, _p)

import numpy as np
import ml_dtypes

import concourse.tile as tile
from concourse import bacc, mybir
from concourse.bass_utils import run_bass_kernel_spmd

TEMPERATURE = 0.07
B = 4096     # simclr batch
D = 256      # projection dim
N = 8192     # num cells (spatial table rows, also 2B simclr table rows)
P = 4096     # num spatial pairs
NCORES = 8
SR = B // NCORES          # 512 simclr pair-rows per core (=> 1024 sim rows)
PR = P // NCORES          # 512 spatial rows per core
RT_SIMCLR = (2 * SR) // 128   # 8 row-tiles
RT_SPATIAL = PR // 128        # 4 row-tiles
RT_TOTAL = RT_SIMCLR + RT_SPATIAL  # 12
NCHUNK = N // 512         # 16 column chunks of 512
NGROUP = 4                # psum groups of 2048 columns
F32 = mybir.dt.float32
BF16 = mybir.dt.bfloat16

_CACHE = {}


def _build_nc():
    nc = bacc.Bacc("TRN2", target_bir_lowering=False)

    zT = nc.dram_tensor("zT", [128, 2, N], BF16, kind="ExternalInput")
    eT = nc.dram_tensor("eT", [128, 2, N], BF16, kind="ExternalInput")
    zTl = nc.dram_tensor("zTl", [128, 2, 2 * SR], BF16, kind="ExternalInput")
    aTl = nc.dram_tensor("aTl", [128, 2, PR], BF16, kind="ExternalInput")
    ident = nc.dram_tensor("ident", [128, 128], F32, kind="ExternalInput")

    sraw_o = nc.dram_tensor("sraw", [128, RT_TOTAL], F32, kind="ExternalOutput")
    corr_o = nc.dram_tensor("corr", [128, RT_TOTAL], F32, kind="ExternalOutput")

    inv_t = float(1.0 / np.float32(TEMPERATURE))

    with tile.TileContext(nc) as tc:
        with (
            tc.tile_pool(name="tabs", bufs=1) as tabs,
            tc.tile_pool(name="psum", bufs=2, space="PSUM") as psum,
            tc.tile_pool(name="scr", bufs=2) as scrp,
            tc.tile_pool(name="small", bufs=1) as small,
            tc.tile_pool(name="tmp", bufs=4) as tmpp,
        ):
            # Small operand tables first so PE can start (grams) immediately;
            # big tables split per 2048-column group so the first main matmul
            # group only waits on its own 1MB chunk.
            zTl_t = tabs.tile([128, 2, 2 * SR], BF16)
            aTl_t = tabs.tile([128, 2, PR], BF16)
            ident_t = small.tile([128, 128], F32)
            zT_g = [tabs.tile([128, 2, 2048], BF16, name=f"zTg{g}")
                    for g in range(NGROUP)]
            eT_g = [tabs.tile([128, 2, 2048], BF16, name=f"eTg{g}")
                    for g in range(NGROUP)]
            # Load order = consumption order: lhsT slices, then the first rhs
            # group (critical path of the first matmul group), then the rest.
            nc.sync.dma_start(zTl_t[:], zTl[:])
            nc.sync.dma_start(aTl_t[:], aTl[:])
            nc.sync.dma_start(zT_g[0][:], zT[:, :, 0:2048])
            nc.sync.dma_start(ident_t[:], ident[:])
            for g in range(1, NGROUP):
                nc.sync.dma_start(zT_g[g][:], zT[:, :, g * 2048:(g + 1) * 2048])
            for g in range(NGROUP):
                nc.sync.dma_start(eT_g[g][:], eT[:, :, g * 2048:(g + 1) * 2048])

            sraw_t = small.tile([128, RT_TOTAL], F32)
            corr_t = small.tile([128, RT_TOTAL], F32)

            def lhsT_pair(rt):
                lh, li = (zTl_t, rt) if rt < RT_SIMCLR else (aTl_t, rt - RT_SIMCLR)
                return (lh[:, 0, li * 128:(li + 1) * 128],
                        lh[:, 1, li * 128:(li + 1) * 128])

            # All Gram diagonals up front, packed into one PSUM tile. The
            # diagonal of lhsT.T@lhsT is bitwise-identical to the main
            # matmul's self-similarity element for each row.
            pgr = psum.tile([128, 2048], F32, tag="big")
            for rt in range(RT_TOTAL):
                l0, l1 = lhsT_pair(rt)
                nc.tensor.matmul(pgr[:, rt * 128:(rt + 1) * 128], l0, l0,
                                 start=True, stop=False)
                nc.tensor.matmul(pgr[:, rt * 128:(rt + 1) * 128], l1, l1,
                                 start=False, stop=True)
            gd_all = tmpp.tile([128, RT_TOTAL, 128], F32, tag="gd")
            for rt in range(RT_TOTAL):
                nc.vector.tensor_tensor(
                    gd_all[:, rt, :], pgr[:, rt * 128:(rt + 1) * 128],
                    ident_t[:], mybir.AluOpType.mult,
                )

            for rt in range(RT_TOTAL):
                lhsT0, lhsT1 = lhsT_pair(rt)
                tab_g = zT_g if rt < RT_SIMCLR else eT_g

                part = tmpp.tile([128, NGROUP], F32, tag="part")
                for g in range(NGROUP):
                    pg = psum.tile([128, 2048], F32, tag="big")
                    for kc, lz in ((0, lhsT0), (1, lhsT1)):
                        for cc in range(4):
                            nc.tensor.matmul(
                                pg[:, cc * 512:(cc + 1) * 512],
                                lz,
                                tab_g[g][:, kc, cc * 512:(cc + 1) * 512],
                                start=(kc == 0),
                                stop=(kc == 1),
                            )
                    scr = scrp.tile([128, 2048], F32, tag="scr")
                    nc.scalar.activation(
                        scr[:], pg[:], mybir.ActivationFunctionType.Exp,
                        scale=inv_t, accum_out=part[:, g:g + 1],
                    )
                nc.vector.tensor_reduce(
                    sraw_t[:, rt:rt + 1], part[:],
                    axis=mybir.AxisListType.X, op=mybir.AluOpType.add,
                )

            # Gram epilogue off ACT's critical path.
            gdv_all = tmpp.tile([128, RT_TOTAL], F32, tag="gdv")
            nc.vector.tensor_reduce(
                gdv_all[:], gd_all[:],
                axis=mybir.AxisListType.X, op=mybir.AluOpType.add,
            )
            nc.scalar.activation(
                corr_t[:], gdv_all[:],
                mybir.ActivationFunctionType.Exp, scale=inv_t,
            )

            nc.sync.dma_start(sraw_o[:], sraw_t[:])
            nc.sync.dma_start(corr_o[:], corr_t[:])

    nc.finalize()
    return nc


def _l2norm(x):
    n = np.maximum(np.linalg.norm(x.astype(np.float32), axis=1, keepdims=True), 1e-12)
    return (x.astype(np.float32) / n).astype(np.float32)


def _pack_T(x):
    """[R, D=256] fp32 -> transposed bf16 operand table [128, 2, R]."""
    xT = np.ascontiguousarray(x.T)                      # [256, R]
    return np.ascontiguousarray(
        xT.reshape(2, 128, xT.shape[1]).transpose(1, 0, 2)
    ).astype(ml_dtypes.bfloat16)


def prepare(z1, z2, embeddings, anchor_idx, neighbor_idx):
    """Host-side prep: returns (in_maps, host_ctx)."""
    z1n = _l2norm(np.asarray(z1))
    z2n = _l2norm(np.asarray(z2))
    en = _l2norm(np.asarray(embeddings))
    ai = np.asarray(anchor_idx).astype(np.int64)
    ni = np.asarray(neighbor_idx).astype(np.int64)

    zcat = np.concatenate([z1n, z2n], axis=0)           # [2B, D]
    zT_p = _pack_T(zcat)                                # [128, 2, 8192] bf16
    eT_p = _pack_T(en)                                  # [128, 2, 8192] bf16
    a_rows = en[ai]                                     # [P, D] fp32
    aT_p = _pack_T(a_rows)                              # [128, 2, 4096] bf16

    # fp32 positive-pair logits (match reference semantics)
    psim = (np.sum(z1n.astype(np.float64) * z2n.astype(np.float64), axis=1)
            / np.float64(np.float32(TEMPERATURE)))      # [B]
    pos = (np.sum(a_rows.astype(np.float64) * en[ni].astype(np.float64), axis=1)
           / np.float64(np.float32(TEMPERATURE)))       # [P]
    eq = (ai == ni).astype(np.float64)                  # [P]

    ident = np.eye(128, dtype=np.float32)
    in_maps = []
    for c in range(NCORES):
        zTl_p = np.ascontiguousarray(np.concatenate(
            [zT_p[:, :, c * SR:(c + 1) * SR],
             zT_p[:, :, B + c * SR:B + (c + 1) * SR]], axis=2))  # [128,2,1024]
        aTl_p = np.ascontiguousarray(aT_p[:, :, c * PR:(c + 1) * PR])  # [128,2,512]
        in_maps.append({
            "zT": zT_p, "eT": eT_p, "zTl": zTl_p, "aTl": aTl_p, "ident": ident,
        })
    return in_maps, (psim, pos, eq)


def finish(results, host_ctx):
    """Host-side epilogue: assemble the two losses from per-core S_raw/corr."""
    psim, pos, eq = host_ctx
    terms1 = np.empty(2 * B, dtype=np.float64)
    terms2 = np.empty(P, dtype=np.float64)
    for c in range(NCORES):
        S = results[c]["sraw"].astype(np.float64).T.reshape(-1)   # [12*128], idx rt*128+p
        C = results[c]["corr"].astype(np.float64).T.reshape(-1)

        s_sim = S[:2 * SR * 1]  # first 8 tiles = 1024 rows
        c_sim = C[:2 * SR]
        sum_exp = s_sim[:2 * SR] - c_sim[:2 * SR]
        p_loc = psim[c * SR:(c + 1) * SR]
        # local rows [0,512) -> z1 part, [512,1024) -> z2 part; same positives
        terms1[c * SR:(c + 1) * SR] = np.log(sum_exp[:SR]) - p_loc
        terms1[B + c * SR:B + (c + 1) * SR] = np.log(sum_exp[SR:2 * SR]) - p_loc

        s_sp = S[2 * SR:2 * SR + PR]
        c_sp = C[2 * SR:2 * SR + PR]
        g = slice(c * PR, (c + 1) * PR)
        total = s_sp - c_sp + eq[g] * np.exp(pos[g])
        terms2[g] = np.log(total) - pos[g]

    l1 = terms1.mean()
    l2 = terms2.mean()
    return np.array([l1, l2], dtype=np.float32)


def get_nc():
    if "nc" not in _CACHE:
        _CACHE["nc"] = _build_nc()
    return _CACHE["nc"]


def kernel(z1, z2, embeddings, anchor_idx, neighbor_idx):
    in_maps, host_ctx = prepare(z1, z2, embeddings, anchor_idx, neighbor_idx)
    nc = get_nc()
    res = run_bass_kernel_spmd(nc, in_maps, list(range(NCORES)))
    return finish(res.results, host_ctx)


# revision 16
# speedup vs baseline: 1.2660x; 1.0011x over previous
"""Trainium2 Bass kernel for nn_ContrastiveLoss (SimCLR + spatial contrastive loss).

Strategy (8-core data parallel):
  - Host: L2-normalize z1/z2/embeddings (fp32), build transposed bf16 operand
    tables, gather anchor rows, compute fp32 positive-pair dots.
  - Device (per core): bf16 matmuls of its 1024 simclr rows and 512 spatial
    rows against the full 8192-column tables; fused exp(x/T) + row-sum on the
    ACT engine straight out of PSUM; a tiny PE Gram matmul per row-tile whose
    diagonal reproduces bit-exactly the self-similarity terms, which are
    exp'd identically and returned as per-row corrections.
  - Host: sum_exp = S_raw - corr (exact diagonal/anchor-column removal),
    log, subtract positives, mean-reduce -> [2] losses.

Self-contained: hardcodes shapes from the problem spec.
"""
import sys

for _p in ("/opt/trn_rl_repo", "/root/.axon_site/_ro/trn_rl_repo"):
    if _p not in sys.path:
        sys.path.insert(0, _p)

import numpy as np
import ml_dtypes

import concourse.tile as tile
from concourse import bacc, mybir
from concourse.bass_utils import run_bass_kernel_spmd

TEMPERATURE = 0.07
B = 4096     # simclr batch
D = 256      # projection dim
N = 8192     # num cells (spatial table rows, also 2B simclr table rows)
P = 4096     # num spatial pairs
NCORES = 8
SR = B // NCORES          # 512 simclr pair-rows per core (=> 1024 sim rows)
PR = P // NCORES          # 512 spatial rows per core
RT_SIMCLR = (2 * SR) // 128   # 8 row-tiles
RT_SPATIAL = PR // 128        # 4 row-tiles
RT_TOTAL = RT_SIMCLR + RT_SPATIAL  # 12
NCHUNK = N // 512         # 16 column chunks of 512
NGROUP = 4                # psum groups of 2048 columns
F32 = mybir.dt.float32
BF16 = mybir.dt.bfloat16

_CACHE = {}


def _build_nc():
    nc = bacc.Bacc("TRN2", target_bir_lowering=False)

    zT = nc.dram_tensor("zT", [128, 2, N], BF16, kind="ExternalInput")
    eT = nc.dram_tensor("eT", [128, 2, N], BF16, kind="ExternalInput")
    zTl = nc.dram_tensor("zTl", [128, 2, 2 * SR], BF16, kind="ExternalInput")
    aTl = nc.dram_tensor("aTl", [128, 2, PR], BF16, kind="ExternalInput")
    ident = nc.dram_tensor("ident", [128, 128], F32, kind="ExternalInput")

    sraw_o = nc.dram_tensor("sraw", [128, RT_TOTAL], F32, kind="ExternalOutput")
    corr_o = nc.dram_tensor("corr", [128, RT_TOTAL], F32, kind="ExternalOutput")

    inv_t = float(1.0 / np.float32(TEMPERATURE))

    with tile.TileContext(nc) as tc:
        with (
            tc.tile_pool(name="tabs", bufs=1) as tabs,
            tc.tile_pool(name="psum", bufs=2, space="PSUM") as psum,
            tc.tile_pool(name="scr", bufs=2) as scrp,
            tc.tile_pool(name="small", bufs=1) as small,
            tc.tile_pool(name="tmp", bufs=4) as tmpp,
        ):
            # Small operand tables first so PE can start (grams) immediately;
            # big tables split per 2048-column group so the first main matmul
            # group only waits on its own 1MB chunk.
            zTl_t = tabs.tile([128, 2, 2 * SR], BF16)
            aTl_t = tabs.tile([128, 2, PR], BF16)
            ident_t = small.tile([128, 128], F32)
            # Group 0 of zT lives in four 512-column sub-tiles so the very
            # first matmuls/exps only wait for 0.25MB of DMA, not 3.2MB.
            zT_c = [tabs.tile([128, 2, 512], BF16, name=f"zTc{j}")
                    for j in range(4)]
            zT_g = [None] + [tabs.tile([128, 2, 2048], BF16, name=f"zTg{g}")
                             for g in range(1, NGROUP)]
            eT_g = [tabs.tile([128, 2, 2048], BF16, name=f"eTg{g}")
                    for g in range(NGROUP)]
            # Load order = consumption order: lhsT slices, then the first rhs
            # chunks (critical path of the first matmul group), then the rest.
            nc.sync.dma_start(zTl_t[:], zTl[:])
            nc.sync.dma_start(aTl_t[:], aTl[:])
            for j in range(4):
                nc.sync.dma_start(zT_c[j][:], zT[:, :, j * 512:(j + 1) * 512])
            nc.sync.dma_start(ident_t[:], ident[:])
            for g in range(1, NGROUP):
                nc.sync.dma_start(zT_g[g][:], zT[:, :, g * 2048:(g + 1) * 2048])
            for g in range(NGROUP):
                nc.sync.dma_start(eT_g[g][:], eT[:, :, g * 2048:(g + 1) * 2048])

            sraw_t = small.tile([128, RT_TOTAL], F32)
            corr_t = small.tile([128, RT_TOTAL], F32)

            def lhsT_pair(rt):
                lh, li = (zTl_t, rt) if rt < RT_SIMCLR else (aTl_t, rt - RT_SIMCLR)
                return (lh[:, 0, li * 128:(li + 1) * 128],
                        lh[:, 1, li * 128:(li + 1) * 128])

            # All Gram diagonals up front: the diagonal of lhsT.T@lhsT is
            # bitwise-identical to the main matmul's self-similarity element
            # for each row; exp'd identically it cancels those terms exactly.
            pgr = psum.tile([128, 2048], F32, tag="big")
            for grt in range(RT_TOTAL):
                l0, l1 = lhsT_pair(grt)
                nc.tensor.matmul(pgr[:, grt * 128:(grt + 1) * 128],
                                 l0, l0, start=True, stop=False)
                nc.tensor.matmul(pgr[:, grt * 128:(grt + 1) * 128],
                                 l1, l1, start=False, stop=True)
            gd_all = tmpp.tile([128, RT_TOTAL, 128], F32, tag="gd")
            for grt in range(RT_TOTAL):
                nc.vector.tensor_tensor(
                    gd_all[:, grt, :],
                    pgr[:, grt * 128:(grt + 1) * 128],
                    ident_t[:], mybir.AluOpType.mult,
                )

            for rt in range(RT_TOTAL):
                lhsT0, lhsT1 = lhsT_pair(rt)
                simclr = rt < RT_SIMCLR
                # rt 0 g 0 runs chunk-at-a-time (512-wide exp) to start ACT as
                # early as possible; everything else uses 2048-wide exps.
                fine_g0 = rt == 0
                nparts = NGROUP + (3 if fine_g0 else 0)

                part = tmpp.tile([128, NGROUP + 3], F32, tag="part")
                pslot = 0
                for g in range(NGROUP):
                    pg = psum.tile([128, 2048], F32, tag="big")
                    if fine_g0 and g == 0:
                        mm_order = [(kc, cc) for cc in range(4) for kc in range(2)]
                    else:
                        mm_order = [(kc, cc) for kc in range(2) for cc in range(4)]
                    for kc, cc in mm_order:
                        lz = lhsT0 if kc == 0 else lhsT1
                        if simclr and g == 0:
                            rhs = zT_c[cc][:, kc, :]
                        else:
                            tab = zT_g[g] if simclr else eT_g[g]
                            rhs = tab[:, kc, cc * 512:(cc + 1) * 512]
                        nc.tensor.matmul(
                            pg[:, cc * 512:(cc + 1) * 512],
                            lz, rhs,
                            start=(kc == 0),
                            stop=(kc == 1),
                        )
                    scr = scrp.tile([128, 2048], F32, tag="scr")
                    if fine_g0 and g == 0:
                        for cc in range(4):
                            nc.scalar.activation(
                                scr[:, cc * 512:(cc + 1) * 512],
                                pg[:, cc * 512:(cc + 1) * 512],
                                mybir.ActivationFunctionType.Exp,
                                scale=inv_t, accum_out=part[:, pslot:pslot + 1],
                            )
                            pslot += 1
                    else:
                        nc.scalar.activation(
                            scr[:], pg[:], mybir.ActivationFunctionType.Exp,
                            scale=inv_t, accum_out=part[:, pslot:pslot + 1],
                        )
                        pslot += 1
                nc.vector.tensor_reduce(
                    sraw_t[:, rt:rt + 1], part[:, :nparts],
                    axis=mybir.AxisListType.X, op=mybir.AluOpType.add,
                )

            gdv_all = tmpp.tile([128, RT_TOTAL], F32, tag="gdv")
            nc.vector.tensor_reduce(
                gdv_all[:], gd_all[:],
                axis=mybir.AxisListType.X, op=mybir.AluOpType.add,
            )
            nc.scalar.activation(
                corr_t[:], gdv_all[:],
                mybir.ActivationFunctionType.Exp, scale=inv_t,
            )

            nc.sync.dma_start(sraw_o[:], sraw_t[:])
            nc.sync.dma_start(corr_o[:], corr_t[:])

    nc.finalize()
    return nc


def _l2norm(x):
    n = np.maximum(np.linalg.norm(x.astype(np.float32), axis=1, keepdims=True), 1e-12)
    return (x.astype(np.float32) / n).astype(np.float32)


def _pack_T(x):
    """[R, D=256] fp32 -> transposed bf16 operand table [128, 2, R]."""
    xT = np.ascontiguousarray(x.T)                      # [256, R]
    return np.ascontiguousarray(
        xT.reshape(2, 128, xT.shape[1]).transpose(1, 0, 2)
    ).astype(ml_dtypes.bfloat16)


def prepare(z1, z2, embeddings, anchor_idx, neighbor_idx):
    """Host-side prep: returns (in_maps, host_ctx)."""
    z1n = _l2norm(np.asarray(z1))
    z2n = _l2norm(np.asarray(z2))
    en = _l2norm(np.asarray(embeddings))
    ai = np.asarray(anchor_idx).astype(np.int64)
    ni = np.asarray(neighbor_idx).astype(np.int64)

    zcat = np.concatenate([z1n, z2n], axis=0)           # [2B, D]
    zT_p = _pack_T(zcat)                                # [128, 2, 8192] bf16
    eT_p = _pack_T(en)                                  # [128, 2, 8192] bf16
    a_rows = en[ai]                                     # [P, D] fp32
    aT_p = _pack_T(a_rows)                              # [128, 2, 4096] bf16

    # fp32 positive-pair logits (match reference semantics)
    psim = (np.sum(z1n.astype(np.float64) * z2n.astype(np.float64), axis=1)
            / np.float64(np.float32(TEMPERATURE)))      # [B]
    pos = (np.sum(a_rows.astype(np.float64) * en[ni].astype(np.float64), axis=1)
           / np.float64(np.float32(TEMPERATURE)))       # [P]
    eq = (ai == ni).astype(np.float64)                  # [P]

    ident = np.eye(128, dtype=np.float32)
    in_maps = []
    for c in range(NCORES):
        zTl_p = np.ascontiguousarray(np.concatenate(
            [zT_p[:, :, c * SR:(c + 1) * SR],
             zT_p[:, :, B + c * SR:B + (c + 1) * SR]], axis=2))  # [128,2,1024]
        aTl_p = np.ascontiguousarray(aT_p[:, :, c * PR:(c + 1) * PR])  # [128,2,512]
        in_maps.append({
            "zT": zT_p, "eT": eT_p, "zTl": zTl_p, "aTl": aTl_p, "ident": ident,
        })
    return in_maps, (psim, pos, eq)


def finish(results, host_ctx):
    """Host-side epilogue: assemble the two losses from per-core S_raw/corr."""
    psim, pos, eq = host_ctx
    terms1 = np.empty(2 * B, dtype=np.float64)
    terms2 = np.empty(P, dtype=np.float64)
    for c in range(NCORES):
        S = results[c]["sraw"].astype(np.float64).T.reshape(-1)   # [12*128], idx rt*128+p
        C = results[c]["corr"].astype(np.float64).T.reshape(-1)

        s_sim = S[:2 * SR * 1]  # first 8 tiles = 1024 rows
        c_sim = C[:2 * SR]
        sum_exp = s_sim[:2 * SR] - c_sim[:2 * SR]
        p_loc = psim[c * SR:(c + 1) * SR]
        # local rows [0,512) -> z1 part, [512,1024) -> z2 part; same positives
        terms1[c * SR:(c + 1) * SR] = np.log(sum_exp[:SR]) - p_loc
        terms1[B + c * SR:B + (c + 1) * SR] = np.log(sum_exp[SR:2 * SR]) - p_loc

        s_sp = S[2 * SR:2 * SR + PR]
        c_sp = C[2 * SR:2 * SR + PR]
        g = slice(c * PR, (c + 1) * PR)
        total = s_sp - c_sp + eq[g] * np.exp(pos[g])
        terms2[g] = np.log(total) - pos[g]

    l1 = terms1.mean()
    l2 = terms2.mean()
    return np.array([l1, l2], dtype=np.float32)


def get_nc():
    if "nc" not in _CACHE:
        _CACHE["nc"] = _build_nc()
    return _CACHE["nc"]


def kernel(z1, z2, embeddings, anchor_idx, neighbor_idx):
    in_maps, host_ctx = prepare(z1, z2, embeddings, anchor_idx, neighbor_idx)
    nc = get_nc()
    res = run_bass_kernel_spmd(nc, in_maps, list(range(NCORES)))
    return finish(res.results, host_ctx)


# revision 17
# speedup vs baseline: 1.2879x; 1.0173x over previous
"""Trainium2 Bass kernel for nn_ContrastiveLoss (SimCLR + spatial contrastive loss).

Strategy (8-core data parallel):
  - Host: L2-normalize z1/z2/embeddings (fp32), build transposed bf16 operand
    tables, gather anchor rows, compute fp32 positive-pair dots.
  - Device (per core): bf16 matmuls of its 1024 simclr rows and 512 spatial
    rows against the full 8192-column tables; fused exp(x/T) + row-sum on the
    ACT engine straight out of PSUM; a tiny PE Gram matmul per row-tile whose
    diagonal reproduces bit-exactly the self-similarity terms, which are
    exp'd identically and returned as per-row corrections.
  - Host: sum_exp = S_raw - corr (exact diagonal/anchor-column removal),
    log, subtract positives, mean-reduce -> [2] losses.

Self-contained: hardcodes shapes from the problem spec.
"""
import sys

for _p in ("/opt/trn_rl_repo", "/root/.axon_site/_ro/trn_rl_repo"):
    if _p not in sys.path:
        sys.path.insert(0, _p)

import numpy as np
import ml_dtypes

import concourse.tile as tile
from concourse import bacc, mybir
from concourse.bass_utils import run_bass_kernel_spmd

TEMPERATURE = 0.07
B = 4096     # simclr batch
D = 256      # projection dim
N = 8192     # num cells (spatial table rows, also 2B simclr table rows)
P = 4096     # num spatial pairs
NCORES = 8
SR = B // NCORES          # 512 simclr pair-rows per core (=> 1024 sim rows)
PR = P // NCORES          # 512 spatial rows per core
RT_SIMCLR = (2 * SR) // 128   # 8 row-tiles
RT_SPATIAL = PR // 128        # 4 row-tiles
RT_TOTAL = RT_SIMCLR + RT_SPATIAL  # 12
NCHUNK = N // 512         # 16 column chunks of 512
NGROUP = 4                # psum groups of 2048 columns
F32 = mybir.dt.float32
BF16 = mybir.dt.bfloat16

_CACHE = {}


def _build_nc():
    nc = bacc.Bacc("TRN2", target_bir_lowering=False)

    zT = nc.dram_tensor("zT", [128, 2, N], BF16, kind="ExternalInput")
    eT = nc.dram_tensor("eT", [128, 2, N], BF16, kind="ExternalInput")
    zTl = nc.dram_tensor("zTl", [128, 2, 2 * SR], BF16, kind="ExternalInput")
    aTl = nc.dram_tensor("aTl", [128, 2, PR], BF16, kind="ExternalInput")
    ident = nc.dram_tensor("ident", [128, 128], F32, kind="ExternalInput")

    sraw_o = nc.dram_tensor("sraw", [128, RT_TOTAL], F32, kind="ExternalOutput")
    corr_o = nc.dram_tensor("corr", [128, RT_TOTAL], F32, kind="ExternalOutput")

    inv_t = float(1.0 / np.float32(TEMPERATURE))

    with tile.TileContext(nc) as tc:
        with (
            tc.tile_pool(name="tabs", bufs=1) as tabs,
            tc.tile_pool(name="psum", bufs=2, space="PSUM") as psum,
            tc.tile_pool(name="scr", bufs=2) as scrp,
            tc.tile_pool(name="small", bufs=1) as small,
            tc.tile_pool(name="tmp", bufs=4) as tmpp,
        ):
            # Small operand tables first so PE can start (grams) immediately;
            # big tables split per 2048-column group so the first main matmul
            # group only waits on its own 1MB chunk.
            zTl_t = tabs.tile([128, 2, 2 * SR], BF16)
            aTl_t = tabs.tile([128, 2, PR], BF16)
            ident_t = small.tile([128, 128], F32)
            # Group 0 of zT lives in four 512-column sub-tiles so the very
            # first matmuls/exps only wait for 0.25MB of DMA, not 3.2MB.
            zT_c = [tabs.tile([128, 2, 512], BF16, name=f"zTc{j}")
                    for j in range(4)]
            zT_g = [None] + [tabs.tile([128, 2, 2048], BF16, name=f"zTg{g}")
                             for g in range(1, NGROUP)]
            eT_g = [tabs.tile([128, 2, 2048], BF16, name=f"eTg{g}")
                    for g in range(NGROUP)]
            # Load order = consumption order: lhsT slices, then the first rhs
            # chunks (critical path of the first matmul group), then the rest.
            nc.sync.dma_start(zTl_t[:], zTl[:])
            nc.sync.dma_start(aTl_t[:], aTl[:])
            for j in range(4):
                nc.sync.dma_start(zT_c[j][:], zT[:, :, j * 512:(j + 1) * 512])
            nc.sync.dma_start(ident_t[:], ident[:])
            for g in range(1, NGROUP):
                nc.sync.dma_start(zT_g[g][:], zT[:, :, g * 2048:(g + 1) * 2048])
            for g in range(NGROUP):
                nc.sync.dma_start(eT_g[g][:], eT[:, :, g * 2048:(g + 1) * 2048])

            sraw_t = small.tile([128, RT_TOTAL], F32)
            corr_t = small.tile([128, RT_TOTAL], F32)

            def lhsT_pair(rt):
                lh, li = (zTl_t, rt) if rt < RT_SIMCLR else (aTl_t, rt - RT_SIMCLR)
                return (lh[:, 0, li * 128:(li + 1) * 128],
                        lh[:, 1, li * 128:(li + 1) * 128])

            # All Gram diagonals up front: the diagonal of lhsT.T@lhsT is
            # bitwise-identical to the main matmul's self-similarity element
            # for each row; exp'd identically it cancels those terms exactly.
            pgr = psum.tile([128, 2048], F32, tag="big")
            for grt in range(RT_TOTAL):
                l0, l1 = lhsT_pair(grt)
                nc.tensor.matmul(pgr[:, grt * 128:(grt + 1) * 128],
                                 l0, l0, start=True, stop=False)
                nc.tensor.matmul(pgr[:, grt * 128:(grt + 1) * 128],
                                 l1, l1, start=False, stop=True)
            gd_all = tmpp.tile([128, RT_TOTAL, 128], F32, tag="gd")
            for grt in range(RT_TOTAL):
                nc.vector.tensor_tensor(
                    gd_all[:, grt, :],
                    pgr[:, grt * 128:(grt + 1) * 128],
                    ident_t[:], mybir.AluOpType.mult,
                )

            for rt in range(RT_TOTAL):
                lhsT0, lhsT1 = lhsT_pair(rt)
                simclr = rt < RT_SIMCLR
                # rt 0 g 0 runs chunk-at-a-time (512-wide exp) to start ACT as
                # early as possible; everything else uses 2048-wide exps.
                fine_g0 = rt == 0
                nparts = NGROUP + (3 if fine_g0 else 0)

                part = tmpp.tile([128, NGROUP + 3], F32, tag="part")
                pslot = 0
                for g in range(NGROUP):
                    pg = psum.tile([128, 2048], F32, tag="big")
                    if fine_g0 and g == 0:
                        mm_order = [(kc, cc) for cc in range(4) for kc in range(2)]
                    else:
                        mm_order = [(kc, cc) for kc in range(2) for cc in range(4)]
                    for kc, cc in mm_order:
                        lz = lhsT0 if kc == 0 else lhsT1
                        if simclr and g == 0:
                            rhs = zT_c[cc][:, kc, :]
                        else:
                            tab = zT_g[g] if simclr else eT_g[g]
                            rhs = tab[:, kc, cc * 512:(cc + 1) * 512]
                        nc.tensor.matmul(
                            pg[:, cc * 512:(cc + 1) * 512],
                            lz, rhs,
                            start=(kc == 0),
                            stop=(kc == 1),
                        )
                    # exp output is dead (only accum_out matters): write it
                    # in-place over the PSUM bank -- PSUM access is cheaper
                    # than an SBUF store on ACT and no scratch tile is needed.
                    if fine_g0 and g == 0:
                        for cc in range(4):
                            nc.scalar.activation(
                                pg[:, cc * 512:(cc + 1) * 512],
                                pg[:, cc * 512:(cc + 1) * 512],
                                mybir.ActivationFunctionType.Exp,
                                scale=inv_t, accum_out=part[:, pslot:pslot + 1],
                            )
                            pslot += 1
                    else:
                        nc.scalar.activation(
                            pg[:], pg[:], mybir.ActivationFunctionType.Exp,
                            scale=inv_t, accum_out=part[:, pslot:pslot + 1],
                        )
                        pslot += 1
                nc.vector.tensor_reduce(
                    sraw_t[:, rt:rt + 1], part[:, :nparts],
                    axis=mybir.AxisListType.X, op=mybir.AluOpType.add,
                )

            gdv_all = tmpp.tile([128, RT_TOTAL], F32, tag="gdv")
            nc.vector.tensor_reduce(
                gdv_all[:], gd_all[:],
                axis=mybir.AxisListType.X, op=mybir.AluOpType.add,
            )
            nc.scalar.activation(
                corr_t[:], gdv_all[:],
                mybir.ActivationFunctionType.Exp, scale=inv_t,
            )

            nc.sync.dma_start(sraw_o[:], sraw_t[:])
            nc.sync.dma_start(corr_o[:], corr_t[:])

    nc.finalize()
    return nc


def _l2norm(x):
    n = np.maximum(np.linalg.norm(x.astype(np.float32), axis=1, keepdims=True), 1e-12)
    return (x.astype(np.float32) / n).astype(np.float32)


def _pack_T(x):
    """[R, D=256] fp32 -> transposed bf16 operand table [128, 2, R]."""
    xT = np.ascontiguousarray(x.T)                      # [256, R]
    return np.ascontiguousarray(
        xT.reshape(2, 128, xT.shape[1]).transpose(1, 0, 2)
    ).astype(ml_dtypes.bfloat16)


def prepare(z1, z2, embeddings, anchor_idx, neighbor_idx):
    """Host-side prep: returns (in_maps, host_ctx)."""
    z1n = _l2norm(np.asarray(z1))
    z2n = _l2norm(np.asarray(z2))
    en = _l2norm(np.asarray(embeddings))
    ai = np.asarray(anchor_idx).astype(np.int64)
    ni = np.asarray(neighbor_idx).astype(np.int64)

    zcat = np.concatenate([z1n, z2n], axis=0)           # [2B, D]
    zT_p = _pack_T(zcat)                                # [128, 2, 8192] bf16
    eT_p = _pack_T(en)                                  # [128, 2, 8192] bf16
    a_rows = en[ai]                                     # [P, D] fp32
    aT_p = _pack_T(a_rows)                              # [128, 2, 4096] bf16

    # fp32 positive-pair logits (match reference semantics)
    psim = (np.sum(z1n.astype(np.float64) * z2n.astype(np.float64), axis=1)
            / np.float64(np.float32(TEMPERATURE)))      # [B]
    pos = (np.sum(a_rows.astype(np.float64) * en[ni].astype(np.float64), axis=1)
           / np.float64(np.float32(TEMPERATURE)))       # [P]
    eq = (ai == ni).astype(np.float64)                  # [P]

    ident = np.eye(128, dtype=np.float32)
    in_maps = []
    for c in range(NCORES):
        zTl_p = np.ascontiguousarray(np.concatenate(
            [zT_p[:, :, c * SR:(c + 1) * SR],
             zT_p[:, :, B + c * SR:B + (c + 1) * SR]], axis=2))  # [128,2,1024]
        aTl_p = np.ascontiguousarray(aT_p[:, :, c * PR:(c + 1) * PR])  # [128,2,512]
        in_maps.append({
            "zT": zT_p, "eT": eT_p, "zTl": zTl_p, "aTl": aTl_p, "ident": ident,
        })
    return in_maps, (psim, pos, eq)


def finish(results, host_ctx):
    """Host-side epilogue: assemble the two losses from per-core S_raw/corr."""
    psim, pos, eq = host_ctx
    terms1 = np.empty(2 * B, dtype=np.float64)
    terms2 = np.empty(P, dtype=np.float64)
    for c in range(NCORES):
        S = results[c]["sraw"].astype(np.float64).T.reshape(-1)   # [12*128], idx rt*128+p
        C = results[c]["corr"].astype(np.float64).T.reshape(-1)

        s_sim = S[:2 * SR * 1]  # first 8 tiles = 1024 rows
        c_sim = C[:2 * SR]
        sum_exp = s_sim[:2 * SR] - c_sim[:2 * SR]
        p_loc = psim[c * SR:(c + 1) * SR]
        # local rows [0,512) -> z1 part, [512,1024) -> z2 part; same positives
        terms1[c * SR:(c + 1) * SR] = np.log(sum_exp[:SR]) - p_loc
        terms1[B + c * SR:B + (c + 1) * SR] = np.log(sum_exp[SR:2 * SR]) - p_loc

        s_sp = S[2 * SR:2 * SR + PR]
        c_sp = C[2 * SR:2 * SR + PR]
        g = slice(c * PR, (c + 1) * PR)
        total = s_sp - c_sp + eq[g] * np.exp(pos[g])
        terms2[g] = np.log(total) - pos[g]

    l1 = terms1.mean()
    l2 = terms2.mean()
    return np.array([l1, l2], dtype=np.float32)


def get_nc():
    if "nc" not in _CACHE:
        _CACHE["nc"] = _build_nc()
    return _CACHE["nc"]


def kernel(z1, z2, embeddings, anchor_idx, neighbor_idx):
    in_maps, host_ctx = prepare(z1, z2, embeddings, anchor_idx, neighbor_idx)
    nc = get_nc()
    res = run_bass_kernel_spmd(nc, in_maps, list(range(NCORES)))
    return finish(res.results, host_ctx)
